# revision 16
# baseline (speedup 1.0000x reference)
"""Trainium2 Bass kernel for nn_MinimalRNNCell.

Reference math (fp32):
    z_t = W_in x_t + b_in
    u_t = sigmoid(Wg_h h_{t-1} + Wg_z z_t + b_g)
    h_t = u_t * h_{t-1} + (1-u_t) * z_t
    y_t = W_out h_t + b_out
    output = y[:, batch=-1, :]  -> [T, O]   (only batch element 63 matters!)

Strategy:
  * Only sample 63 of the batch affects the output -> compute just that one.
  * The gated recurrence is a contraction (u in (0,1)); influence of the
    starting state decays ~0.5^k.  Split T=4096 into chunks that restart
    from h=0 with a W=16-step warmup; chunking error is far below the fp16
    rounding of the matmul datapath (~2.4e-5 total vs the fp32 reference).
  * 8 cores each own 512 contiguous timesteps, split into C=64 parallel
    "lanes" of L=8 steps (+W warmup) batched in the matmul free dim, so
    each core runs only S = W+L = 24 sequential steps.
  * Per step only THREE ops sit on the serial critical path
    (matmul -> sigmoid -> vector-mult):
        m_t = u_t * d_t              (m = h - z, the "decaying part")
        d_t = m_{t-1} + (z_{t-1} - z_t)           [off critical path]
        pre_t = Wg_h m_{t-1} + P2_t, P2_t = Wg_z z_t + Wg_h z_{t-1}
    P2 is precomputed in bulk and injected into PSUM with identity-weight
    matmuls, so the accumulating Wg_h matmul is the only serial PE op.
  * All matmuls use fp16 operands (fp32 on the TRN2 PE needs LOW/HIGH
    double passes at 4 cyc/row).  Bulk matmuls (Z, P2, y) use hi/lo fp16
    pairs (Dekker splitting) for ~fp32 accuracy; the recurrence state m
    stays fp32 (a second vector-mult emits the fp16 copy fed to the PE).
  * Dummy matmuls at kernel start warm the PE HAM clock gate (1.2->2.4GHz)
    while the input DMAs are in flight; inputs are packed into 4 DMAs.
"""

import numpy as np

import concourse.bass as bass
import concourse.mybir as mybir
import concourse.tile as tile
from concourse import bacc
from concourse.bass_utils import run_bass_kernel_spmd

# problem constants (hardcoded per harness contract)
T, I, H, O = 4096, 64, 128, 64
NCORES = 8
TLOC = T // NCORES          # timesteps per core
W = 16                      # warmup steps per lane
C = 64                      # lanes per core
L = TLOC // C               # real steps per lane
S = W + L                   # sequential steps per core
NZ = 1 + W + TLOC           # z columns per core (1 leading col for z_{t-1})

# fp16 const-blob column layout: [wgh_h | wgh_l | wgz_h | wgz_l | ident |
#                                 wout_h | wout_l | win_h | win_l]
_C16_COLS = {
    "wgh_h": (0, 128), "wgh_l": (128, 128), "wgz_h": (256, 128),
    "wgz_l": (384, 128), "ident": (512, 128), "wout_h": (640, 64),
    "wout_l": (704, 64), "win_h": (768, 128), "win_l": (896, 128),
}
NC16 = 1024
# fp32 const-blob: [bin | bg | bout(64) | zmask(W+1)]
NC32 = 2 + 64 + (W + 1)

FP32 = mybir.dt.float32
FP16 = mybir.dt.float16
AF = mybir.ActivationFunctionType


def _build_program():
    nc = bacc.Bacc()

    xT_h = nc.dram_tensor("xT_h", [I, NZ], FP16, kind="ExternalInput")
    xT_l = nc.dram_tensor("xT_l", [I, NZ], FP16, kind="ExternalInput")
    c16 = nc.dram_tensor("c16", [128, NC16], FP16, kind="ExternalInput")
    c32 = nc.dram_tensor("c32", [128, NC32], FP32, kind="ExternalInput")
    y = nc.dram_tensor("y", [TLOC, O], FP32, kind="ExternalOutput")

    with tile.TileContext(nc) as tc:
        with (
            tc.tile_pool(name="singles", bufs=1) as singles,
            tc.tile_pool(name="state", bufs=5) as state,
            tc.tile_pool(name="psum_big", bufs=2, space="PSUM") as psum_big,
            tc.tile_pool(name="psum_u", bufs=4, space="PSUM") as psum_u_pool,
            tc.tile_pool(name="psum_y", bufs=2, space="PSUM") as psum_y_pool,
        ):
            # ---- PE warm-up: dummy matmuls engage the HAM fast clock while
            # the input DMAs are still in flight ----
            junk = singles.tile([128, 512], FP16)
            nc.vector.memset(junk, 0.0)
            ps_junk = psum_y_pool.tile([128, 512], FP32, tag="ps_y")
            for _ in range(12):
                nc.tensor.matmul(ps_junk, junk[:, 0:128], junk,
                                 start=True, stop=True, skip_group_check=True)

            # ---- load inputs (4 DMAs, split across 2 queues) ----
            xh_sb = singles.tile([I, NZ], FP16)
            xl_sb = singles.tile([I, NZ], FP16)
            c16_sb = singles.tile([128, NC16], FP16)
            c32_sb = singles.tile([128, NC32], FP32)
            nc.sync.dma_start(out=xh_sb, in_=xT_h[:, :])
            nc.gpsimd.dma_start(out=xl_sb, in_=xT_l[:, :])
            nc.gpsimd.dma_start(out=c16_sb, in_=c16[:, :])
            nc.sync.dma_start(out=c32_sb, in_=c32[:, :])

            def c16s(name, rows=128):
                c0, n = _C16_COLS[name]
                return c16_sb[0:rows, c0:c0 + n]

            wghh_sb = c16s("wgh_h")
            wghl_sb = c16s("wgh_l")
            wgzh_sb = c16s("wgz_h")
            wgzl_sb = c16s("wgz_l")
            ident_sb = c16s("ident")
            wouth_sb = c16s("wout_h")
            woutl_sb = c16s("wout_l")
            winh_sb = c16s("win_h", rows=I)
            winl_sb = c16s("win_l", rows=I)
            bin_sb = c32_sb[:, 0:1]
            bg_sb = c32_sb[:, 1:2]
            bout_sb = c32_sb[:, 2:66]
            zmask_sb = c32_sb[:, 66:66 + W + 1]

            # ---- Z = W_in @ x + b_in  (fp16-pair matmul) -> Zp fp32 ----
            Zp = singles.tile([H, NZ], FP32)
            blocks = [(0, 512), (512, NZ - 512)]
            for c0, cn in blocks:
                ps = psum_big.tile([H, cn], FP32, tag="ps_big")
                nc.tensor.matmul(ps, winh_sb, xh_sb[:, c0:c0 + cn],
                                 start=True, stop=False)
                nc.tensor.matmul(ps, winh_sb, xl_sb[:, c0:c0 + cn],
                                 start=False, stop=False)
                nc.tensor.matmul(ps, winl_sb, xh_sb[:, c0:c0 + cn],
                                 start=False, stop=True)
                nc.scalar.activation(Zp[:, c0:c0 + cn], ps, AF.Identity,
                                     bias=bin_sb)
            # zero the warmup pad (z must be 0, not b_in, where t<0)
            nc.vector.tensor_mul(Zp[:, 0:W + 1], Zp[:, 0:W + 1], zmask_sb)

            # fp16 hi/lo split of Zp for the P2 bulk matmuls; Delta on gpsimd
            Zh = singles.tile([H, NZ], FP16)
            Zl = singles.tile([H, NZ], FP16)
            nc.vector.tensor_copy(Zh, Zp)
            nc.vector.tensor_sub(Zl, Zp, Zh)
            Delta = singles.tile([H, NZ], FP32)
            nc.gpsimd.tensor_sub(Delta[:, 1:NZ], Zp[:, 0:NZ - 1], Zp[:, 1:NZ])

            # ---- P2[:, u] = Wg_z Zp[:, u] + Wg_h Zp[:, u-1]  (u >= 1) ----
            # hi/lo fp16 split is cast straight from PSUM (no fp32 bounce)
            P2h = singles.tile([H, NZ], FP16)
            P2l = singles.tile([H, NZ], FP16)
            for c0, cn in ((1, 512), (513, NZ - 513)):
                ps = psum_big.tile([H, cn], FP32, tag="ps_big")
                nc.tensor.matmul(ps, wgzh_sb, Zh[:, c0:c0 + cn],
                                 start=True, stop=False)
                nc.tensor.matmul(ps, wgzh_sb, Zl[:, c0:c0 + cn],
                                 start=False, stop=False)
                nc.tensor.matmul(ps, wgzl_sb, Zh[:, c0:c0 + cn],
                                 start=False, stop=False)
                nc.tensor.matmul(ps, wghh_sb, Zh[:, c0 - 1:c0 - 1 + cn],
                                 start=False, stop=False)
                nc.tensor.matmul(ps, wghh_sb, Zl[:, c0 - 1:c0 - 1 + cn],
                                 start=False, stop=False)
                nc.tensor.matmul(ps, wghl_sb, Zh[:, c0 - 1:c0 - 1 + cn],
                                 start=False, stop=True)
                nc.scalar.activation(P2h[:, c0:c0 + cn], ps, AF.Copy)
                nc.vector.tensor_sub(P2l[:, c0:c0 + cn], ps,
                                     P2h[:, c0:c0 + cn])

            # keep the PE HAM clock warm across the cast/DVE gap before the
            # recurrence (an idle window >3.4us would re-throttle to 1.2GHz)
            for _ in range(10):
                nc.tensor.matmul(ps_junk, junk[:, 0:128], junk,
                                 start=True, stop=True, skip_group_check=True)

            # ---- recurrence: 2 independent streams interleaved ----
            NG = 2                # streams per core
            CG = C // NG          # lanes per stream
            GSPAN = (CG - 1) * L + 1
            GOFF = TLOC // NG     # timestep offset of stream 1

            Mhist = singles.tile([H, TLOC], FP32)

            m_prev, m16_prev = [], []
            for g in range(NG):
                mi = state.tile([H, CG], FP32, tag="mscratch")
                nc.vector.memset(mi, 0.0)
                m16i = state.tile([H, CG], FP16, tag="m16")
                nc.vector.memset(m16i, 0.0)
                m_prev.append(mi)
                m16_prev.append(m16i)

            for s in range(S):
                for g in range(NG):
                    base = g * GOFF + s + 1  # column of z_t for lane 0
                    sl = slice(base, base + GSPAN, L)

                    ps_u = psum_u_pool.tile([H, CG], FP32, tag="ps_u")
                    nc.tensor.matmul(ps_u, ident_sb, P2h[:, sl],
                                     start=True, stop=False)
                    nc.tensor.matmul(ps_u, ident_sb, P2l[:, sl],
                                     start=False, stop=False)
                    nc.tensor.matmul(ps_u, wghh_sb, m16_prev[g],
                                     start=False, stop=True)

                    u_s = state.tile([H, CG], FP32, tag="u")
                    nc.scalar.activation(u_s, ps_u, AF.Sigmoid, bias=bg_sb)

                    d_s = state.tile([H, CG], FP32, tag="d")
                    nc.vector.tensor_add(d_s, m_prev[g], Delta[:, sl])

                    m16_s = state.tile([H, CG], FP16, tag="m16")
                    nc.vector.tensor_mul(m16_s, u_s, d_s)

                    if s >= W:
                        c0 = g * GOFF + s - W
                        m_out = Mhist[:, c0:c0 + GSPAN:L]
                    else:
                        m_out = state.tile([H, CG], FP32, tag="mscratch")
                    nc.gpsimd.tensor_mul(m_out, u_s, d_s)
                    m_prev[g], m16_prev[g] = m_out, m16_s

            # ---- h = m + z ; y = h^T W_out^T + b_out (fp16-pair) ----
            Hh = singles.tile([H, TLOC], FP32)
            nc.vector.tensor_add(Hh, Mhist, Zp[:, W + 1:NZ])
            Hhh = singles.tile([H, TLOC], FP16)
            Hhl = singles.tile([H, TLOC], FP16)
            nc.vector.tensor_copy(Hhh, Hh)
            nc.vector.tensor_sub(Hhl, Hh, Hhh)

            ysb = singles.tile([128, TLOC // 128, O], FP32)
            for b in range(TLOC // 128):
                bs = slice(b * 128, (b + 1) * 128)
                ps_y = psum_y_pool.tile([128, O], FP32, tag="ps_y")
                nc.tensor.matmul(ps_y, Hhh[:, bs], wouth_sb,
                                 start=True, stop=False)
                nc.tensor.matmul(ps_y, Hhh[:, bs], woutl_sb,
                                 start=False, stop=False)
                nc.tensor.matmul(ps_y, Hhl[:, bs], wouth_sb,
                                 start=False, stop=True)
                nc.vector.tensor_add(ysb[:, b, :], ps_y, bout_sb)
            y_view = y.rearrange("(b p) o -> p b o", p=128)
            nc.sync.dma_start(out=y_view, in_=ysb)

    nc.compile()
    return nc


_PROGRAM = None


def _get_program():
    global _PROGRAM
    if _PROGRAM is None:
        _PROGRAM = _build_program()
    return _PROGRAM


def _pair16(a):
    hi = a.astype(np.float16)
    lo = (a - hi.astype(np.float32)).astype(np.float16)
    return np.ascontiguousarray(hi), np.ascontiguousarray(lo)


def _prepare_in_maps(inputs):
    x = np.ascontiguousarray(np.asarray(inputs["inputs"], dtype=np.float32)[63])
    W_in = np.asarray(inputs["W_in"], dtype=np.float32)
    b_in = np.asarray(inputs["b_in"], dtype=np.float32)
    W_g = np.asarray(inputs["W_g"], dtype=np.float32)
    b_g = np.asarray(inputs["b_g"], dtype=np.float32)
    W_out = np.asarray(inputs["W_out"], dtype=np.float32)
    b_out = np.asarray(inputs["b_out"], dtype=np.float32)

    Wg_h = W_g[:, :H]
    Wg_z = W_g[:, H:]

    c16 = np.zeros((128, NC16), np.float16)

    def put(name, hi, lo=None, rows=128):
        c0, n = _C16_COLS[name]
        c16[:rows, c0:c0 + n] = hi
        if lo is not None:
            c0l, nl = _C16_COLS[lo[0]]
            c16[:rows, c0l:c0l + nl] = lo[1]

    wghh, wghl = _pair16(Wg_h.T)
    wgzh, wgzl = _pair16(Wg_z.T)
    wouth, woutl = _pair16(W_out.T)
    winh, winl = _pair16(W_in.T)
    put("wgh_h", wghh, ("wgh_l", wghl))
    put("wgz_h", wgzh, ("wgz_l", wgzl))
    put("wout_h", wouth, ("wout_l", woutl))
    put("win_h", winh, ("win_l", winl), rows=I)
    c16[:, 512:640] = np.eye(128, dtype=np.float16)

    c32 = np.zeros((128, NC32), np.float32)
    c32[:, 0] = b_in
    c32[:, 1] = b_g
    c32[:, 2:66] = np.tile(b_out[None, :], (128, 1))

    # x padded on the left with W+1 zero rows (z-space zeros via zmask)
    xpad = np.concatenate([np.zeros((W + 1, I), np.float32), x], axis=0)

    in_maps = []
    for k in range(NCORES):
        lo = k * TLOC
        xk_h, xk_l = _pair16(xpad[lo:lo + NZ].T)
        c32k = c32.copy()
        c32k[:, 66:66 + W + 1] = 0.0 if k == 0 else 1.0
        in_maps.append({"xT_h": xk_h, "xT_l": xk_l, "c16": c16, "c32": c32k})
    return in_maps


def _run(in_maps, **kwargs):
    nc = _get_program()
    return run_bass_kernel_spmd(nc, in_maps, list(range(NCORES)), **kwargs)


def kernel(**inputs):
    res = _run(_prepare_in_maps(inputs))
    y = np.concatenate([res.results[k]["y"] for k in range(NCORES)], axis=0)
    return np.ascontiguousarray(y.astype(np.float32))


if __name__ == "__main__":
    d = np.load("/root/problem/inputs.npz")
    out = kernel(**{k: d[k] for k in d.files})
    exp = np.load("/root/problem/expected.npy")
    err = np.abs(out - exp).max()
    print("absmax err vs expected:", err, " rel:", err / np.abs(exp).max())


# revision 17
# speedup vs baseline: 1.0844x; 1.0844x over previous
"""Trainium2 Bass kernel for nn_MinimalRNNCell.

Reference math (fp32):
    z_t = W_in x_t + b_in
    u_t = sigmoid(Wg_h h_{t-1} + Wg_z z_t + b_g)
    h_t = u_t * h_{t-1} + (1-u_t) * z_t
    y_t = W_out h_t + b_out
    output = y[:, batch=-1, :]  -> [T, O]   (only batch element 63 matters!)

Strategy:
  * Only sample 63 of the batch affects the output -> compute just that one.
  * The gated recurrence is a contraction (u in (0,1)); influence of the
    starting state decays ~0.5^k.  Split T=4096 into chunks that restart
    from h=0 with a W=16-step warmup; chunking error is far below the fp16
    rounding of the matmul datapath (~2.4e-5 total vs the fp32 reference).
  * 8 cores each own 512 contiguous timesteps, split into C=64 parallel
    "lanes" of L=8 steps (+W warmup) batched in the matmul free dim, so
    each core runs only S = W+L = 24 sequential steps.
  * Per step only THREE ops sit on the serial critical path
    (matmul -> sigmoid -> vector-mult):
        m_t = u_t * d_t              (m = h - z, the "decaying part")
        d_t = m_{t-1} + (z_{t-1} - z_t)           [off critical path]
        pre_t = Wg_h m_{t-1} + P2_t, P2_t = Wg_z z_t + Wg_h z_{t-1}
    P2 is precomputed in bulk and injected into PSUM with identity-weight
    matmuls, so the accumulating Wg_h matmul is the only serial PE op.
  * All matmuls use fp16 operands (fp32 on the TRN2 PE needs LOW/HIGH
    double passes at 4 cyc/row).  Bulk matmuls (Z, P2, y) use hi/lo fp16
    pairs (Dekker splitting) for ~fp32 accuracy; the recurrence state m
    stays fp32 (a second vector-mult emits the fp16 copy fed to the PE).
  * Dummy matmuls at kernel start warm the PE HAM clock gate (1.2->2.4GHz)
    while the input DMAs are in flight; inputs are packed into 4 DMAs.
"""

import numpy as np

import concourse.bass as bass
import concourse.mybir as mybir
import concourse.tile as tile
from concourse import bacc
from concourse.bass_utils import run_bass_kernel_spmd

# problem constants (hardcoded per harness contract)
T, I, H, O = 4096, 64, 128, 64
NCORES = 8
TLOC = T // NCORES          # timesteps per core
W = 16                      # warmup steps per lane
C = 64                      # lanes per core
L = TLOC // C               # real steps per lane
S = W + L                   # sequential steps per core
NZ = 1 + W + TLOC           # z columns per core (1 leading col for z_{t-1})

# fp16 const-blob column layout: [wgh_h | wgh_l | wgz_h | wgz_l | ident |
#                                 wout_h | wout_l | win_h | win_l]
_C16_COLS = {
    "wgh_h": (0, 128), "wgh_l": (128, 128), "wgz_h": (256, 128),
    "wgz_l": (384, 128), "ident": (512, 128), "wout_h": (640, 64),
    "wout_l": (704, 64), "win_h": (768, 128), "win_l": (896, 128),
}
NC16 = 1024
# fp32 const-blob: [bin | bg | bout(64) | zmask(W+1)]
NC32 = 2 + 64 + (W + 1)

FP32 = mybir.dt.float32
FP16 = mybir.dt.float16
AF = mybir.ActivationFunctionType


def _build_program():
    nc = bacc.Bacc()

    xT_h = nc.dram_tensor("xT_h", [I, NZ], FP16, kind="ExternalInput")
    xT_l = nc.dram_tensor("xT_l", [I, NZ], FP16, kind="ExternalInput")
    c16 = nc.dram_tensor("c16", [128, NC16], FP16, kind="ExternalInput")
    c32 = nc.dram_tensor("c32", [128, NC32], FP32, kind="ExternalInput")
    y = nc.dram_tensor("y", [TLOC, O], FP32, kind="ExternalOutput")

    with tile.TileContext(nc) as tc:
        with (
            tc.tile_pool(name="singles", bufs=1) as singles,
            tc.tile_pool(name="state", bufs=3) as state,
            tc.tile_pool(name="psum_big", bufs=2, space="PSUM") as psum_big,
            tc.tile_pool(name="psum_u", bufs=2, space="PSUM") as psum_u_pool,
            tc.tile_pool(name="psum_uo", bufs=2, space="PSUM") as psum_uo_pool,
            tc.tile_pool(name="psum_y", bufs=2, space="PSUM") as psum_y_pool,
        ):
            # ---- PE warm-up: dummy matmuls engage the HAM fast clock while
            # the input DMAs are still in flight ----
            junk = singles.tile([128, 512], FP16)
            nc.vector.memset(junk, 0.0)
            ps_junk = psum_y_pool.tile([128, 512], FP32, tag="ps_y")
            for _ in range(12):
                nc.tensor.matmul(ps_junk, junk[:, 0:128], junk,
                                 start=True, stop=True, skip_group_check=True)

            # ---- load inputs (4 DMAs, split across 2 queues) ----
            xh_sb = singles.tile([I, NZ], FP16)
            xl_sb = singles.tile([I, NZ], FP16)
            c16_sb = singles.tile([128, NC16], FP16)
            c32_sb = singles.tile([128, NC32], FP32)
            nc.sync.dma_start(out=xh_sb, in_=xT_h[:, :])
            nc.gpsimd.dma_start(out=xl_sb, in_=xT_l[:, :])
            nc.gpsimd.dma_start(out=c16_sb, in_=c16[:, :])
            nc.sync.dma_start(out=c32_sb, in_=c32[:, :])

            def c16s(name, rows=128):
                c0, n = _C16_COLS[name]
                return c16_sb[0:rows, c0:c0 + n]

            wghh_sb = c16s("wgh_h")
            wghl_sb = c16s("wgh_l")
            wgzh_sb = c16s("wgz_h")
            wgzl_sb = c16s("wgz_l")
            ident_sb = c16s("ident")
            wouth_sb = c16s("wout_h")
            woutl_sb = c16s("wout_l")
            winh_sb = c16s("win_h", rows=I)
            winl_sb = c16s("win_l", rows=I)
            bin_sb = c32_sb[:, 0:1]
            bg_sb = c32_sb[:, 1:2]
            bout_sb = c32_sb[:, 2:66]
            zmask_sb = c32_sb[:, 66:66 + W + 1]

            # ---- Z = W_in @ x + b_in  (fp16-pair matmul) -> Zp fp32 ----
            Zp = singles.tile([H, NZ], FP32)
            blocks = [(0, 512), (512, NZ - 512)]
            for c0, cn in blocks:
                ps = psum_big.tile([H, cn], FP32, tag="ps_big")
                nc.tensor.matmul(ps, winh_sb, xh_sb[:, c0:c0 + cn],
                                 start=True, stop=False)
                nc.tensor.matmul(ps, winh_sb, xl_sb[:, c0:c0 + cn],
                                 start=False, stop=False)
                nc.tensor.matmul(ps, winl_sb, xh_sb[:, c0:c0 + cn],
                                 start=False, stop=True)
                nc.scalar.activation(Zp[:, c0:c0 + cn], ps, AF.Identity,
                                     bias=bin_sb)
            # zero the warmup pad (z must be 0, not b_in, where t<0)
            nc.vector.tensor_mul(Zp[:, 0:W + 1], Zp[:, 0:W + 1], zmask_sb)

            # fp16 hi/lo split of Zp for the P2 bulk matmuls; Delta on gpsimd
            Zh = singles.tile([H, NZ], FP16)
            Zl = singles.tile([H, NZ], FP16)
            nc.vector.tensor_copy(Zh, Zp)
            nc.vector.tensor_sub(Zl, Zp, Zh)
            Delta = singles.tile([H, NZ], FP32)
            nc.gpsimd.tensor_sub(Delta[:, 1:NZ], Zp[:, 0:NZ - 1], Zp[:, 1:NZ])

            # ---- P2[:, u] = Wg_z Zp[:, u] + Wg_h Zp[:, u-1]  (u >= 1) ----
            # hi/lo fp16 split is cast straight from PSUM (no fp32 bounce)
            P2h = singles.tile([H, NZ], FP16)
            P2l = singles.tile([H, NZ], FP16)
            for c0, cn in ((1, 512), (513, NZ - 513)):
                ps = psum_big.tile([H, cn], FP32, tag="ps_big")
                nc.tensor.matmul(ps, wgzh_sb, Zh[:, c0:c0 + cn],
                                 start=True, stop=False)
                nc.tensor.matmul(ps, wgzh_sb, Zl[:, c0:c0 + cn],
                                 start=False, stop=False)
                nc.tensor.matmul(ps, wgzl_sb, Zh[:, c0:c0 + cn],
                                 start=False, stop=False)
                nc.tensor.matmul(ps, wghh_sb, Zh[:, c0 - 1:c0 - 1 + cn],
                                 start=False, stop=False)
                nc.tensor.matmul(ps, wghh_sb, Zl[:, c0 - 1:c0 - 1 + cn],
                                 start=False, stop=False)
                nc.tensor.matmul(ps, wghl_sb, Zh[:, c0 - 1:c0 - 1 + cn],
                                 start=False, stop=True)
                nc.scalar.activation(P2h[:, c0:c0 + cn], ps, AF.Copy)
                nc.vector.tensor_sub(P2l[:, c0:c0 + cn], ps,
                                     P2h[:, c0:c0 + cn])

            # keep the PE HAM clock warm across the cast/DVE gap before the
            # recurrence (an idle window >3.4us would re-throttle to 1.2GHz)
            for _ in range(10):
                nc.tensor.matmul(ps_junk, junk[:, 0:128], junk,
                                 start=True, stop=True, skip_group_check=True)

            # ---- recurrence ----
            Mhist = singles.tile([H, TLOC], FP32)
            span = (C - 1) * L + 1  # strided-slice span over lanes

            m_init = state.tile([H, C], FP32, tag="mscratch")
            nc.vector.memset(m_init, 0.0)
            m16_init = state.tile([H, C], FP16, tag="m16")
            nc.vector.memset(m16_init, 0.0)
            m_prev, m16_prev = m_init, m16_init

            for s in range(S):
                base = s + 1  # column of z_t for lane 0
                sl = slice(base, base + span, L)

                ps_u = psum_u_pool.tile([H, C], FP32, tag="ps_u")
                nc.tensor.matmul(ps_u, ident_sb, P2h[:, sl],
                                 start=True, stop=False)
                nc.tensor.matmul(ps_u, ident_sb, P2l[:, sl],
                                 start=False, stop=False)
                nc.tensor.matmul(ps_u, wghh_sb, m16_prev,
                                 start=False, stop=True)

                u_s = state.tile([H, C], FP32, tag="u")
                nc.scalar.activation(u_s, ps_u, AF.Sigmoid, bias=bg_sb)

                d_s = state.tile([H, C], FP32, tag="d")
                nc.vector.tensor_add(d_s, m_prev, Delta[:, sl])

                m16_s = state.tile([H, C], FP16, tag="m16")
                nc.vector.tensor_mul(m16_s, u_s, d_s)

                if s >= W:
                    m_out = Mhist[:, s - W:s - W + span:L]
                else:
                    m_out = state.tile([H, C], FP32, tag="mscratch")
                nc.vector.tensor_mul(m_out, u_s, d_s)
                m_prev, m16_prev = m_out, m16_s

            # ---- h = m + z ; y = h^T W_out^T + b_out (fp16-pair) ----
            Hh = singles.tile([H, TLOC], FP32)
            nc.vector.tensor_add(Hh, Mhist, Zp[:, W + 1:NZ])
            Hhh = singles.tile([H, TLOC], FP16)
            Hhl = singles.tile([H, TLOC], FP16)
            nc.vector.tensor_copy(Hhh, Hh)
            nc.vector.tensor_sub(Hhl, Hh, Hhh)

            ysb = singles.tile([128, TLOC // 128, O], FP32)
            for b in range(TLOC // 128):
                bs = slice(b * 128, (b + 1) * 128)
                ps_y = psum_y_pool.tile([128, O], FP32, tag="ps_y")
                nc.tensor.matmul(ps_y, Hhh[:, bs], wouth_sb,
                                 start=True, stop=False)
                nc.tensor.matmul(ps_y, Hhh[:, bs], woutl_sb,
                                 start=False, stop=False)
                nc.tensor.matmul(ps_y, Hhl[:, bs], wouth_sb,
                                 start=False, stop=True)
                nc.vector.tensor_add(ysb[:, b, :], ps_y, bout_sb)
            y_view = y.rearrange("(b p) o -> p b o", p=128)
            nc.sync.dma_start(out=y_view, in_=ysb)

    nc.compile()
    return nc


_PROGRAM = None


def _get_program():
    global _PROGRAM
    if _PROGRAM is None:
        _PROGRAM = _build_program()
    return _PROGRAM


def _pair16(a):
    hi = a.astype(np.float16)
    lo = (a - hi.astype(np.float32)).astype(np.float16)
    return np.ascontiguousarray(hi), np.ascontiguousarray(lo)


def _prepare_in_maps(inputs):
    x = np.ascontiguousarray(np.asarray(inputs["inputs"], dtype=np.float32)[63])
    W_in = np.asarray(inputs["W_in"], dtype=np.float32)
    b_in = np.asarray(inputs["b_in"], dtype=np.float32)
    W_g = np.asarray(inputs["W_g"], dtype=np.float32)
    b_g = np.asarray(inputs["b_g"], dtype=np.float32)
    W_out = np.asarray(inputs["W_out"], dtype=np.float32)
    b_out = np.asarray(inputs["b_out"], dtype=np.float32)

    Wg_h = W_g[:, :H]
    Wg_z = W_g[:, H:]

    c16 = np.zeros((128, NC16), np.float16)

    def put(name, hi, lo=None, rows=128):
        c0, n = _C16_COLS[name]
        c16[:rows, c0:c0 + n] = hi
        if lo is not None:
            c0l, nl = _C16_COLS[lo[0]]
            c16[:rows, c0l:c0l + nl] = lo[1]

    wghh, wghl = _pair16(Wg_h.T)
    wgzh, wgzl = _pair16(Wg_z.T)
    wouth, woutl = _pair16(W_out.T)
    winh, winl = _pair16(W_in.T)
    put("wgh_h", wghh, ("wgh_l", wghl))
    put("wgz_h", wgzh, ("wgz_l", wgzl))
    put("wout_h", wouth, ("wout_l", woutl))
    put("win_h", winh, ("win_l", winl), rows=I)
    c16[:, 512:640] = np.eye(128, dtype=np.float16)

    c32 = np.zeros((128, NC32), np.float32)
    c32[:, 0] = b_in
    c32[:, 1] = b_g
    c32[:, 2:66] = np.tile(b_out[None, :], (128, 1))

    # x padded on the left with W+1 zero rows (z-space zeros via zmask)
    xpad = np.concatenate([np.zeros((W + 1, I), np.float32), x], axis=0)

    in_maps = []
    for k in range(NCORES):
        lo = k * TLOC
        xk_h, xk_l = _pair16(xpad[lo:lo + NZ].T)
        c32k = c32.copy()
        c32k[:, 66:66 + W + 1] = 0.0 if k == 0 else 1.0
        in_maps.append({"xT_h": xk_h, "xT_l": xk_l, "c16": c16, "c32": c32k})
    return in_maps


def _run(in_maps, **kwargs):
    nc = _get_program()
    return run_bass_kernel_spmd(nc, in_maps, list(range(NCORES)), **kwargs)


def kernel(**inputs):
    res = _run(_prepare_in_maps(inputs))
    y = np.concatenate([res.results[k]["y"] for k in range(NCORES)], axis=0)
    return np.ascontiguousarray(y.astype(np.float32))


if __name__ == "__main__":
    d = np.load("/root/problem/inputs.npz")
    out = kernel(**{k: d[k] for k in d.files})
    exp = np.load("/root/problem/expected.npy")
    err = np.abs(out - exp).max()
    print("absmax err vs expected:", err, " rel:", err / np.abs(exp).max())
